# revision 4
# baseline (speedup 1.0000x reference)
"""Trainium2 Bass kernel for: out = 0.5 * sum_g maxpool4(x @ W.T + b).

Shapes: x [4096, 2048] f32, W [4096, 2048] f32, b [4096] f32 -> out [4096] f32.

Sharding over 8 NeuronCores: 4 batch-groups x 2 out-feature-groups.
Core c = (g, j): batch rows g*1024:(g+1)*1024, out features j*2048:(j+1)*2048.
Each core computes partial row-sums of its pooled half; host adds the 2
out-feature partials per batch quarter (pooling groups of 4 never split).

Per-core kernel, fp8 e4m3 + DoubleRow (the key change vs the bf16 version):
the PE array virtualizes to 128x256 - each matmul contracts over 256 rows
(a k-pair: two 128-blocks in the AP's middle dim), so the matmul stream halves
to 256 instructions of ~241 ns vs 512 x 216 ns for bf16. Inputs quantize to
e4m3 on host; PSUM accumulation stays fp32, so the only error is input
quantization (~5% per y element), which dilutes to <0.5% in the final
1024-term pooled row-sum (measured 4.9e-3 vs the 2e-2 gate). W and b are
pre-scaled by 16 = 0.5*32 (pow2, exact) to lift W out of the e4m3 subnormal
range; the host divides the final output by 32.

Layout: stationary lhsT = x^T k-pair slice [128p, 2, 128b] (LDWEIGHTS hides
under the 4 matmuls that reuse it - measured on the bf16 baseline: LDW issues
~2 ns after the preceding matmul and overlaps on the separate SBUF port).
Moving rhs = W^T k-pair slice [128p, 2, 512o] -> PSUM [128b, 512o] fp32.
4 PSUM banks per batch tile, double-buffered across the 8 batch tiles, so the
VectorE epilogue (bias add + maxpool4 + row-sum, ~4.2 us/tile) overlaps the
next tile's ~7.7 us matmul span. Last batch tile runs ot-outer so its pooling
overlaps its own later matmuls, shrinking the tail.

DMA: W^T slabs (512 KiB each) alternate Sync/Scalar queues, x^T slabs on
GpSimd, bias on Vector - all streams run in parallel and stay ahead of the
k-pair-ordered consumption. First k-pair is split into fine pieces so the
first matmul starts right after the NEFF preamble.
"""

import sys

if "/opt/trn_rl_repo" not in sys.path:
    sys.path.insert(0, "/opt/trn_rl_repo")

import numpy as np
import ml_dtypes

# Problem constants (hardcoded per harness contract).
B, I, O = 4096, 2048, 4096
KS = 4  # maxpool kernel size
NB_G, NO_G = 4, 2  # batch groups x out-feature groups = 8 cores
BC = B // NB_G  # 1024 batch rows per core
OC = O // NO_G  # 2048 out features per core
P = 128
KP = I // (2 * P)  # 8 k-pair slabs (256 contraction rows each)
BT = BC // P  # 8 batch tiles per core
NFREE = 512  # matmul moving free dim (one PSUM bank fp32)
OT = OC // NFREE  # 4 out-feature tiles of 512 per core
WB_SCALE = 16.0  # 0.5 (model scale) * 32 folded into W,b; host divides by 32

_NC_CACHE = {}


def _dedup_ldweights(nc):
    """Remove redundant standalone Ldweights from the compiled module.

    bacc splits every Matmult into Ldweights + Matmult(ldweights=False) with
    no dedup. When consecutive PE Ldweights load the identical stationary AP,
    the array already holds the weights, so sync-free duplicates can be
    dropped (a wait-carrying Ldweights guards a real dependency).
    """
    removed = 0
    for f in nc.m.functions:
        for blk in f.blocks:
            insts = list(blk.instructions)
            keep = []
            blk_removed = 0
            last_key = None
            for ins in insts:
                tname = type(ins).__name__
                if tname == "InstLdweights":
                    ap = ins.ins[0]
                    key = (
                        ap.memref,
                        ap.offset,
                        str(ap.ap),
                        str(ap.dtype),
                        str(ins.tile_position),
                        str(ins.tile_size),
                        str(ins.perf_mode),
                        str(ins.is_transpose),
                    )
                    if (
                        key == last_key
                        and not ins.has_wait()
                        and not ins.has_update()
                    ):
                        blk_removed += 1
                        continue
                    last_key = key
                keep.append(ins)
            if blk_removed:
                blk.instructions[:] = keep
                removed += blk_removed
    return removed


def build_bass():
    """Build the (SPMD, per-core) Bass program."""
    from concourse import bacc, tile
    import concourse.mybir as mybir

    f32 = mybir.dt.float32
    bf16 = mybir.dt.bfloat16
    fp8 = mybir.dt.float8e4
    DR = mybir.MatmulPerfMode.DoubleRow

    nc = bacc.Bacc(
        "TRN2",
        target_bir_lowering=False,
        debug=False,
        num_devices=NB_G * NO_G,
        enable_asserts=False,
        num_swdge_queues=2,
    )
    xt_d = nc.dram_tensor("xt", [KP, P, 2, BC], fp8, kind="ExternalInput")
    wt_d = nc.dram_tensor("wt", [KP, P, 2, OC], fp8, kind="ExternalInput")
    biasrep_d = nc.dram_tensor("biasrep", [P, OC], bf16, kind="ExternalInput")
    out_d = nc.dram_tensor("out", [P, BT], f32, kind="ExternalOutput")

    with tile.TileContext(nc) as tc:
        with (
            tc.tile_pool(name="wt", bufs=KP) as wt_pool,
            tc.tile_pool(name="xt", bufs=KP) as xt_pool,
            tc.tile_pool(name="misc", bufs=1) as misc_pool,
            tc.tile_pool(name="tsum", bufs=4) as tsum_pool,
            tc.tile_pool(name="pooled", bufs=2) as pooled_pool,
            tc.tile_pool(name="psum", bufs=8, space="PSUM") as psum_pool,
        ):
            # k-pair 0 arrives in fine pieces so the first matmul (gated
            # behind the ~6.5 us NEFF preamble) starts after ~160 KB; the
            # pieces gating it ride different engine queues so their DMA
            # first-byte latencies overlap.
            w_sb = []
            x_sb = []
            w0 = wt_pool.tile([P, 2, OC], fp8, tag="wt", name="w_0")
            x0 = xt_pool.tile([P, 2, BC], fp8, tag="xt", name="x_0")
            nc.scalar.dma_start(x0[:, :, 0:P], xt_d[0, :, :, 0:P])
            nc.sync.dma_start(w0[:, :, 0:NFREE], wt_d[0, :, :, 0:NFREE])
            nc.gpsimd.dma_start(x0[:, :, P:], xt_d[0, :, :, P:])
            nc.scalar.dma_start(
                w0[:, :, NFREE : 2 * NFREE], wt_d[0, :, :, NFREE : 2 * NFREE]
            )
            nc.sync.dma_start(w0[:, :, 2 * NFREE :], wt_d[0, :, :, 2 * NFREE :])
            w_sb.append(w0)
            x_sb.append(x0)
            for kp in range(1, KP):
                w = wt_pool.tile([P, 2, OC], fp8, tag="wt", name=f"w_{kp}")
                q = nc.sync if kp % 2 else nc.scalar
                q.dma_start(w[:], wt_d[kp, :, :, :])
                w_sb.append(w)
                x = xt_pool.tile([P, 2, BC], fp8, tag="xt", name=f"x_{kp}")
                nc.gpsimd.dma_start(x[:], xt_d[kp, :, :, :])
                x_sb.append(x)

            biasrep = misc_pool.tile([P, OC], bf16)
            nc.gpsimd.dma_start(biasrep[:], biasrep_d[:, :])
            outsb = misc_pool.tile([P, BT], f32)

            def emit_pooling(bt, psums, pooled_t, ots):
                # Adds first (each frees its PSUM bank), then the maxpools.
                tsums = []
                for ot in ots:
                    ts_ = tsum_pool.tile(
                        [P, NFREE], bf16, tag="tsum", name=f"tsum_{bt}_{ot}"
                    )
                    nc.vector.tensor_add(
                        ts_[:],
                        psums[ot][:],
                        biasrep[:, ot * NFREE : (ot + 1) * NFREE],
                    )
                    tsums.append(ts_)
                for ot, ts_ in zip(ots, tsums):
                    nc.vector.reduce_max(
                        pooled_t[:, ot, :],
                        ts_[:].rearrange("p (q f) -> p q f", f=KS),
                        axis=mybir.AxisListType.X,
                    )

            for bt in range(BT):
                pooled_t = pooled_pool.tile(
                    [P, OT, P], bf16, tag="pooled", name=f"pooled_{bt}"
                )
                psums = [
                    psum_pool.tile([P, NFREE], f32, tag="ps", name=f"ps_{bt}_{ot}")
                    for ot in range(OT)
                ]
                if bt < BT - 1:
                    # k-pair-major: each W slab is consumed by 4 back-to-back
                    # matmuls the moment it lands; pooling at the tile tail
                    # overlaps the next tile's matmuls on the other 4 banks.
                    for kp in range(KP):
                        lhsT = x_sb[kp][:, :, bt * P : (bt + 1) * P]
                        for ot in range(OT):
                            nc.tensor.matmul(
                                psums[ot][:],
                                lhsT,
                                w_sb[kp][:, :, ot * NFREE : (ot + 1) * NFREE],
                                start=(kp == 0),
                                stop=(kp == KP - 1),
                                perf_mode=DR,
                            )
                        if kp == KP - 1:
                            emit_pooling(bt, psums, pooled_t, range(OT))
                else:
                    # Last tile: ot-outer so each bank's pooling overlaps the
                    # later ot's matmuls instead of piling up after the end
                    # (everything is SBUF-resident by now).
                    for ot in range(OT):
                        lhs_bt = slice(bt * P, (bt + 1) * P)
                        for kp in range(KP):
                            nc.tensor.matmul(
                                psums[ot][:],
                                x_sb[kp][:, :, lhs_bt],
                                w_sb[kp][:, :, ot * NFREE : (ot + 1) * NFREE],
                                start=(kp == 0),
                                stop=(kp == KP - 1),
                                perf_mode=DR,
                            )
                        emit_pooling(bt, psums, pooled_t, [ot])
                nc.vector.reduce_sum(
                    outsb[:, bt : bt + 1],
                    pooled_t[:, :, :],
                    axis=mybir.AxisListType.XY,
                )
            nc.scalar.dma_start(out_d[:, :], outsb[:, :])

    nc.compile()
    _dedup_ldweights(nc)
    return nc


def make_in_maps(x, W, b):
    """Host-side shard + preprocess: transpose, fold 16=0.5*32, cast e4m3."""
    x = np.asarray(x, dtype=np.float32)
    W = np.asarray(W, dtype=np.float32)
    b = np.asarray(b, dtype=np.float32)

    fp8 = ml_dtypes.float8_e4m3

    # Per-batch-group x slabs: [KP, P, 2, BC], k = kp*256 + slot*128 + p.
    x_slabs = []
    for g in range(NB_G):
        xgT = np.ascontiguousarray(x[g * BC : (g + 1) * BC, :].T).astype(fp8)
        x_slabs.append(
            np.ascontiguousarray(
                xgT.reshape(KP, 2, P, BC).transpose(0, 2, 1, 3)
            )
        )
    # Per-out-group W slabs [KP, P, 2, OC] and replicated bias rows.
    w_slabs = []
    b_slabs = []
    for j in range(NO_G):
        wjT = np.ascontiguousarray(
            W[j * OC : (j + 1) * OC, :].T * np.float32(WB_SCALE)
        ).astype(fp8)
        w_slabs.append(
            np.ascontiguousarray(
                wjT.reshape(KP, 2, P, OC).transpose(0, 2, 1, 3)
            )
        )
        bj = (b[j * OC : (j + 1) * OC] * np.float32(WB_SCALE)).astype(
            ml_dtypes.bfloat16
        )
        b_slabs.append(
            np.ascontiguousarray(np.broadcast_to(bj.reshape(1, OC), (P, OC)))
        )

    in_maps = []
    for c in range(NB_G * NO_G):
        g, j = divmod(c, NO_G)
        in_maps.append({"xt": x_slabs[g], "wt": w_slabs[j], "biasrep": b_slabs[j]})
    return in_maps


def combine_outputs(results):
    """Sum the 2 out-feature partials per batch quarter -> full [B] output."""
    out = np.zeros(B, dtype=np.float32)
    for c, r in enumerate(results):
        g = c // NO_G
        part = np.asarray(r["out"], dtype=np.float32)  # [P, BT]
        # batch index within the core = bt*P + p
        out[g * BC : (g + 1) * BC] += part.T.reshape(BC)
    return out * np.float32(1.0 / 32.0)


def kernel(x, W, b):
    from concourse.bass_utils import run_bass_kernel_spmd

    if "nc" not in _NC_CACHE:
        _NC_CACHE["nc"] = build_bass()
    nc = _NC_CACHE["nc"]
    in_maps = make_in_maps(x, W, b)
    res = run_bass_kernel_spmd(nc, in_maps, core_ids=list(range(NB_G * NO_G)))
    return combine_outputs(res.results)


# revision 5
# speedup vs baseline: 1.0421x; 1.0421x over previous
"""Trainium2 Bass kernel for: out = 0.5 * sum_g maxpool4(x @ W.T + b).

Shapes: x [4096, 2048] f32, W [4096, 2048] f32, b [4096] f32 -> out [4096] f32.

Sharding over 8 NeuronCores: 2 batch-groups x 4 out-feature-groups.
Core c = (g, j): batch rows g*2048:(g+1)*2048, out features j*1024:(j+1)*1024.
Each core computes partial row-sums of its pooled quarter; host adds the 4
out-feature partials per batch half (pooling groups of 4 never split).

Per-core kernel, fp8 e4m3 + DoubleRow: the PE array virtualizes to 128x256 -
each matmul contracts over 256 rows (a k-pair, the AP's middle dim=2), so the
matmul stream halves to 256 instructions pacing at ~216 ns vs 512 for bf16.
Inputs quantize to e4m3 on host; PSUM accumulation stays fp32, so the only
error is input quantization (~5% per y element), which dilutes to <0.5% in
the final 1024-term pooled row-sum (measured 4.9e-3 vs the 2e-2 gate). W and
b are pre-scaled by 16 = 0.5*32 (pow2, exact) to lift W out of the e4m3
subnormal range; the host divides the final output by 32.

Layout: stationary lhsT = x^T k-pair slice [128p, 2, 128b] (the LDWEIGHTS
hides under the 2 matmuls reusing it - 1:2 ratio). Moving rhs = W^T k-pair
slice [128p, 2, 512o] -> PSUM [128b, 512o] fp32. 2 PSUM banks per batch tile,
4-deep rotation over the 8 banks, so the VectorE epilogue (bias add +
maxpool4 + row-sum, ~3.1 us/tile) has ~13.8 us of slack per bank reuse.

The out-feature split is 4-way (vs batch 2-way) to keep per-core W at 1 MiB:
the first batch tile burns through ALL of W in ~3.5 us, so W DMA demand peaks
at W_bytes/3.5us - the 4x2 variant of this kernel (W=4 MiB) measured ~11 us
of matmul stalls waiting on W in the first tiles. x is DMA'd in bt-major
slabs so each tile's 8 stationaries arrive as one contiguous 256 KiB block
well ahead of use. W k-pair slabs alternate Sync/Scalar queues; x+bias ride
GpSimd; the first pieces of each are split fine so the first matmul starts
right after the NEFF preamble.
"""

import sys

if "/opt/trn_rl_repo" not in sys.path:
    sys.path.insert(0, "/opt/trn_rl_repo")

import numpy as np
import ml_dtypes

# Problem constants (hardcoded per harness contract).
B, I, O = 4096, 2048, 4096
KS = 4  # maxpool kernel size
NB_G, NO_G = 2, 4  # batch groups x out-feature groups = 8 cores
BC = B // NB_G  # 2048 batch rows per core
OC = O // NO_G  # 1024 out features per core
P = 128
KP = I // (2 * P)  # 8 k-pair slabs (256 contraction rows each)
BT = BC // P  # 16 batch tiles per core
NFREE = 512  # matmul moving free dim (one PSUM bank fp32)
OT = OC // NFREE  # 2 out-feature tiles of 512 per core
WB_SCALE = 16.0  # 0.5 (model scale) * 32 folded into W,b; host divides by 32

_NC_CACHE = {}


def _dedup_ldweights(nc):
    """Remove redundant standalone Ldweights from the compiled module.

    bacc splits every Matmult into Ldweights + Matmult(ldweights=False) with
    no dedup. When consecutive PE Ldweights load the identical stationary AP,
    the array already holds the weights, so sync-free duplicates can be
    dropped (a wait-carrying Ldweights guards a real dependency).
    """
    removed = 0
    for f in nc.m.functions:
        for blk in f.blocks:
            insts = list(blk.instructions)
            keep = []
            blk_removed = 0
            last_key = None
            for ins in insts:
                tname = type(ins).__name__
                if tname == "InstLdweights":
                    ap = ins.ins[0]
                    key = (
                        ap.memref,
                        ap.offset,
                        str(ap.ap),
                        str(ap.dtype),
                        str(ins.tile_position),
                        str(ins.tile_size),
                        str(ins.perf_mode),
                        str(ins.is_transpose),
                    )
                    if (
                        key == last_key
                        and not ins.has_wait()
                        and not ins.has_update()
                    ):
                        blk_removed += 1
                        continue
                    last_key = key
                keep.append(ins)
            if blk_removed:
                blk.instructions[:] = keep
                removed += blk_removed
    return removed


def build_bass():
    """Build the (SPMD, per-core) Bass program."""
    from concourse import bacc, tile
    import concourse.mybir as mybir

    f32 = mybir.dt.float32
    bf16 = mybir.dt.bfloat16
    fp8 = mybir.dt.float8e4
    DR = mybir.MatmulPerfMode.DoubleRow

    nc = bacc.Bacc(
        "TRN2",
        target_bir_lowering=False,
        debug=False,
        num_devices=NB_G * NO_G,
        enable_asserts=False,
        num_swdge_queues=2,
    )
    # x^T in bt-major slabs: [bt, p, kp, slot, m], k = kp*256 + slot*128 + p.
    xt_d = nc.dram_tensor("xt", [BT, P, KP, 2, P], fp8, kind="ExternalInput")
    # W^T in k-pair slabs: [kp, p, slot, o].
    wt_d = nc.dram_tensor("wt", [KP, P, 2, OC], fp8, kind="ExternalInput")
    biasrep_d = nc.dram_tensor("biasrep", [P, OC], bf16, kind="ExternalInput")
    out_d = nc.dram_tensor("out", [P, BT], f32, kind="ExternalOutput")

    with tile.TileContext(nc) as tc:
        with (
            tc.tile_pool(name="wt", bufs=KP) as wt_pool,
            tc.tile_pool(name="xt", bufs=BT) as xt_pool,
            tc.tile_pool(name="misc", bufs=1) as misc_pool,
            tc.tile_pool(name="tsum", bufs=4) as tsum_pool,
            tc.tile_pool(name="pooled", bufs=4) as pooled_pool,
            tc.tile_pool(name="psum", bufs=8, space="PSUM") as psum_pool,
        ):
            # First pieces split fine so the first matmul (gated behind the
            # ~6.5 us NEFF preamble) starts as early as possible; the gating
            # pieces ride different engine queues so their DMA first-byte
            # latencies overlap.
            w_sb = []
            x_sb = []
            w0 = wt_pool.tile([P, 2, OC], fp8, tag="wt", name="w_0")
            x0 = xt_pool.tile([P, KP, 2, P], fp8, tag="xt", name="x_0")
            nc.gpsimd.dma_start(x0[:, 0, :, :], xt_d[0, :, 0, :, :])
            nc.sync.dma_start(w0[:, :, 0:NFREE], wt_d[0, :, :, 0:NFREE])
            nc.gpsimd.dma_start(x0[:, 1:, :, :], xt_d[0, :, 1:, :, :])
            nc.scalar.dma_start(w0[:, :, NFREE:], wt_d[0, :, :, NFREE:])
            w_sb.append(w0)
            x_sb.append(x0)
            for kp in range(1, KP):
                w = wt_pool.tile([P, 2, OC], fp8, tag="wt", name=f"w_{kp}")
                q = nc.sync if kp % 2 == 0 else nc.scalar
                q.dma_start(w[:], wt_d[kp, :, :, :])
                w_sb.append(w)
            biasrep = misc_pool.tile([P, OC], bf16)
            for bt in range(1, BT):
                x = xt_pool.tile([P, KP, 2, P], fp8, tag="xt", name=f"x_{bt}")
                nc.gpsimd.dma_start(x[:], xt_d[bt, :, :, :, :])
                x_sb.append(x)
                if bt == 2:
                    # Bias lands ~3 us in, ahead of its first use at bt0's
                    # epilogue (~4.3 us); earlier would delay x slabs.
                    nc.gpsimd.dma_start(biasrep[:], biasrep_d[:, :])
            outsb = misc_pool.tile([P, BT], f32)

            def emit_pooling(bt, psums, pooled_t, ots):
                # Adds first (each frees its PSUM bank), then the maxpools.
                tsums = []
                for ot in ots:
                    ts_ = tsum_pool.tile(
                        [P, NFREE], bf16, tag="tsum", name=f"tsum_{bt}_{ot}"
                    )
                    nc.vector.tensor_add(
                        ts_[:],
                        psums[ot][:],
                        biasrep[:, ot * NFREE : (ot + 1) * NFREE],
                    )
                    tsums.append(ts_)
                for ot, ts_ in zip(ots, tsums):
                    nc.vector.reduce_max(
                        pooled_t[:, ot, :],
                        ts_[:].rearrange("p (q f) -> p q f", f=KS),
                        axis=mybir.AxisListType.X,
                    )

            for bt in range(BT):
                pooled_t = pooled_pool.tile(
                    [P, OT, P], bf16, tag="pooled", name=f"pooled_{bt}"
                )
                psums = [
                    psum_pool.tile([P, NFREE], f32, tag="ps", name=f"ps_{bt}_{ot}")
                    for ot in range(OT)
                ]
                if bt < BT - 1:
                    # k-pair-major: each W slab is consumed by 2 back-to-back
                    # matmuls; pooling at the tile tail overlaps later tiles'
                    # matmuls on other banks (4-deep bank rotation).
                    for kp in range(KP):
                        lhsT = x_sb[bt][:, kp, :, :]
                        for ot in range(OT):
                            nc.tensor.matmul(
                                psums[ot][:],
                                lhsT,
                                w_sb[kp][:, :, ot * NFREE : (ot + 1) * NFREE],
                                start=(kp == 0),
                                stop=(kp == KP - 1),
                                perf_mode=DR,
                            )
                        if kp == KP - 1:
                            emit_pooling(bt, psums, pooled_t, range(OT))
                else:
                    # Last tile: ot-outer so the first bank's pooling overlaps
                    # the second bank's matmuls, shrinking the tail.
                    for ot in range(OT):
                        for kp in range(KP):
                            nc.tensor.matmul(
                                psums[ot][:],
                                x_sb[bt][:, kp, :, :],
                                w_sb[kp][:, :, ot * NFREE : (ot + 1) * NFREE],
                                start=(kp == 0),
                                stop=(kp == KP - 1),
                                perf_mode=DR,
                            )
                        emit_pooling(bt, psums, pooled_t, [ot])
                nc.vector.reduce_sum(
                    outsb[:, bt : bt + 1],
                    pooled_t[:, :, :],
                    axis=mybir.AxisListType.XY,
                )
            nc.scalar.dma_start(out_d[:, :], outsb[:, :])

    nc.compile()
    _dedup_ldweights(nc)
    return nc


def make_in_maps(x, W, b):
    """Host-side shard + preprocess: transpose, fold 16=0.5*32, cast e4m3."""
    x = np.asarray(x, dtype=np.float32)
    W = np.asarray(W, dtype=np.float32)
    b = np.asarray(b, dtype=np.float32)

    fp8 = ml_dtypes.float8_e4m3

    # Per-batch-group x slabs: [BT, P, KP, 2, P], k = kp*256 + slot*128 + p,
    # batch = bt*128 + m.
    x_slabs = []
    for g in range(NB_G):
        xgT = np.ascontiguousarray(x[g * BC : (g + 1) * BC, :].T).astype(fp8)
        x_slabs.append(
            np.ascontiguousarray(
                xgT.reshape(KP, 2, P, BT, P).transpose(3, 2, 0, 1, 4)
            )
        )
    # Per-out-group W slabs [KP, P, 2, OC] and replicated bias rows.
    w_slabs = []
    b_slabs = []
    for j in range(NO_G):
        wjT = np.ascontiguousarray(
            W[j * OC : (j + 1) * OC, :].T * np.float32(WB_SCALE)
        ).astype(fp8)
        w_slabs.append(
            np.ascontiguousarray(
                wjT.reshape(KP, 2, P, OC).transpose(0, 2, 1, 3)
            )
        )
        bj = (b[j * OC : (j + 1) * OC] * np.float32(WB_SCALE)).astype(
            ml_dtypes.bfloat16
        )
        b_slabs.append(
            np.ascontiguousarray(np.broadcast_to(bj.reshape(1, OC), (P, OC)))
        )

    in_maps = []
    for c in range(NB_G * NO_G):
        g, j = divmod(c, NO_G)
        in_maps.append({"xt": x_slabs[g], "wt": w_slabs[j], "biasrep": b_slabs[j]})
    return in_maps


def combine_outputs(results):
    """Sum the 4 out-feature partials per batch half -> full [B] output."""
    out = np.zeros(B, dtype=np.float32)
    for c, r in enumerate(results):
        g = c // NO_G
        part = np.asarray(r["out"], dtype=np.float32)  # [P, BT]
        # batch index within the core = bt*P + p
        out[g * BC : (g + 1) * BC] += part.T.reshape(BC)
    return out * np.float32(1.0 / 32.0)


def kernel(x, W, b):
    from concourse.bass_utils import run_bass_kernel_spmd

    if "nc" not in _NC_CACHE:
        _NC_CACHE["nc"] = build_bass()
    nc = _NC_CACHE["nc"]
    in_maps = make_in_maps(x, W, b)
    res = run_bass_kernel_spmd(nc, in_maps, core_ids=list(range(NB_G * NO_G)))
    return combine_outputs(res.results)


# revision 10
# speedup vs baseline: 1.0662x; 1.0231x over previous
"""Trainium2 Bass kernel for: out = 0.5 * sum_g maxpool4(x @ W.T + b).

Shapes: x [4096, 2048] f32, W [4096, 2048] f32, b [4096] f32 -> out [4096] f32.

Sharding over 8 NeuronCores: 2 batch-groups x 4 out-feature-groups.
Core c = (g, j): batch rows g*2048:(g+1)*2048, out features j*1024:(j+1)*1024.
Each core computes partial row-sums of its pooled quarter; host adds the 4
out-feature partials per batch half (pooling groups of 4 never split).

Per-core kernel, fp8 e4m3 + DoubleRow: the PE array virtualizes to 128x256 -
each matmul contracts over 256 rows (a k-pair, the AP's middle dim=2), so the
matmul stream halves to 256 instructions pacing at ~216 ns vs 512 for bf16.
Inputs quantize to e4m3 on host; PSUM accumulation stays fp32, so the only
error is input quantization (~5% per y element), which dilutes to <0.5% in
the final 1024-term pooled row-sum (measured 4.9e-3 vs the 2e-2 gate). W and
b are pre-scaled by 16 = 0.5*32 (pow2, exact) to lift W out of the e4m3
subnormal range; the host divides the final output by 32.

Layout: stationary lhsT = x^T k-pair slice [128p, 2, 128b] (the LDWEIGHTS
hides under the 2 matmuls reusing it - 1:2 ratio). Moving rhs = W^T k-pair
slice [128p, 2, 512o] -> PSUM [128b, 512o] fp32. 2 PSUM banks per batch tile,
4-deep rotation over the 8 banks, so the VectorE epilogue (bias add +
maxpool4 + row-sum, ~3.1 us/tile) has ~13.8 us of slack per bank reuse.

The out-feature split is 4-way (vs batch 2-way) to keep per-core W at 1 MiB:
the first batch tile burns through ALL of W in ~3.5 us, so W DMA demand peaks
at W_bytes/3.5us - the 4x2 variant of this kernel (W=4 MiB) measured ~11 us
of matmul stalls waiting on W in the first tiles. x is DMA'd in bt-major
slabs so each tile's 8 stationaries arrive as one contiguous 256 KiB block
well ahead of use. W k-pair slabs alternate Sync/Scalar queues; x+bias ride
GpSimd; the first pieces of each are split fine so the first matmul starts
right after the NEFF preamble.
"""

import sys

if "/opt/trn_rl_repo" not in sys.path:
    sys.path.insert(0, "/opt/trn_rl_repo")

import numpy as np
import ml_dtypes

# Problem constants (hardcoded per harness contract).
B, I, O = 4096, 2048, 4096
KS = 4  # maxpool kernel size
NB_G, NO_G = 2, 4  # batch groups x out-feature groups = 8 cores
BC = B // NB_G  # 2048 batch rows per core
OC = O // NO_G  # 1024 out features per core
P = 128
KP = I // (2 * P)  # 8 k-pair slabs (256 contraction rows each)
BT = BC // P  # 16 batch tiles per core
NFREE = 512  # matmul moving free dim (one PSUM bank fp32)
OT = OC // NFREE  # 2 out-feature tiles of 512 per core
WB_SCALE = 16.0  # 0.5 (model scale) * 32 folded into W,b; host divides by 32

_NC_CACHE = {}


def _dedup_ldweights(nc):
    """Remove redundant standalone Ldweights from the compiled module.

    bacc splits every Matmult into Ldweights + Matmult(ldweights=False) with
    no dedup. When consecutive PE Ldweights load the identical stationary AP,
    the array already holds the weights, so sync-free duplicates can be
    dropped (a wait-carrying Ldweights guards a real dependency).
    """
    removed = 0
    for f in nc.m.functions:
        for blk in f.blocks:
            insts = list(blk.instructions)
            keep = []
            blk_removed = 0
            last_key = None
            for ins in insts:
                tname = type(ins).__name__
                if tname == "InstLdweights":
                    ap = ins.ins[0]
                    key = (
                        ap.memref,
                        ap.offset,
                        str(ap.ap),
                        str(ap.dtype),
                        str(ins.tile_position),
                        str(ins.tile_size),
                        str(ins.perf_mode),
                        str(ins.is_transpose),
                    )
                    if (
                        key == last_key
                        and not ins.has_wait()
                        and not ins.has_update()
                    ):
                        blk_removed += 1
                        continue
                    last_key = key
                keep.append(ins)
            if blk_removed:
                blk.instructions[:] = keep
                removed += blk_removed
    return removed


def build_bass():
    """Build the (SPMD, per-core) Bass program."""
    from concourse import bacc, tile
    import concourse.mybir as mybir

    f32 = mybir.dt.float32
    bf16 = mybir.dt.bfloat16
    fp8 = mybir.dt.float8e4
    DR = mybir.MatmulPerfMode.DoubleRow

    nc = bacc.Bacc(
        "TRN2",
        target_bir_lowering=False,
        debug=False,
        num_devices=NB_G * NO_G,
        enable_asserts=False,
        num_swdge_queues=2,
    )
    # x^T in bt-major slabs: [bt, p, kp, slot, m], k = kp*256 + slot*128 + p.
    xt_d = nc.dram_tensor("xt", [BT, P, KP, 2, P], fp8, kind="ExternalInput")
    # Duplicate of the (bt0, kp0) stationary as its own tensor: consumers
    # wait on ALL DMAs into a tile, so the first LDWEIGHTS gets a dedicated
    # 32 KiB tile instead of waiting for bt0's full 256 KiB slab.
    xt0_d = nc.dram_tensor("xt0", [P, 2, P], fp8, kind="ExternalInput")
    # W^T in k-pair slabs: [kp, p, slot, o].
    wt_d = nc.dram_tensor("wt", [KP, P, 2, OC], fp8, kind="ExternalInput")
    biasrep_d = nc.dram_tensor("biasrep", [P, OC], bf16, kind="ExternalInput")
    out_d = nc.dram_tensor("out", [P, BT], f32, kind="ExternalOutput")

    with tile.TileContext(nc) as tc:
        with (
            tc.tile_pool(name="wt", bufs=KP) as wt_pool,
            tc.tile_pool(name="xt", bufs=BT) as xt_pool,
            tc.tile_pool(name="misc", bufs=1) as misc_pool,
            tc.tile_pool(name="tsum", bufs=4) as tsum_pool,
            tc.tile_pool(name="pooled", bufs=4) as pooled_pool,
            tc.tile_pool(name="psum", bufs=8, space="PSUM") as psum_pool,
        ):
            # All three DMA rings (Sync/Scalar/GpSimd) share the 16 HW DMA
            # engines (~330 GB/s aggregate, ~1/3 per active ring), so issue
            # order is deadline-ordered and byte-balanced per ring: W slabs
            # (consumed at 0.43 us/slab by bt0) split across Sync+Scalar with
            # x3/x2 tucked behind them; x_first/x0/x1/bias on GpSimd; the
            # remaining x slabs trail on GpSimd (needed 3.46 us apart).
            w_sb = []
            x_sb = [None] * BT
            x_first = misc_pool.tile([P, 2, P], fp8)
            nc.gpsimd.dma_start(x_first[:], xt0_d[:, :, :])
            for kp in range(KP):
                w = wt_pool.tile([P, 2, OC], fp8, tag="wt", name=f"w_{kp}")
                q = nc.sync if kp % 2 == 0 else nc.scalar
                q.dma_start(w[:], wt_d[kp, :, :, :])
                w_sb.append(w)

            def x_dma(bt, q):
                x = xt_pool.tile([P, KP, 2, P], fp8, tag="xt", name=f"x_{bt}")
                q.dma_start(x[:], xt_d[bt, :, :, :, :])
                x_sb[bt] = x

            x_dma(0, nc.gpsimd)
            x_dma(1, nc.gpsimd)
            biasrep = misc_pool.tile([P, OC], bf16)
            nc.gpsimd.dma_start(biasrep[:], biasrep_d[:, :])
            x_dma(2, nc.scalar)
            x_dma(3, nc.sync)
            for bt in range(4, BT):
                x_dma(bt, nc.gpsimd)
            outsb = misc_pool.tile([P, BT], f32)

            def emit_pooling(bt, psums, pooled_t, ots):
                # Adds first (each frees its PSUM bank), then the maxpools.
                tsums = []
                for ot in ots:
                    ts_ = tsum_pool.tile(
                        [P, NFREE], bf16, tag="tsum", name=f"tsum_{bt}_{ot}"
                    )
                    nc.vector.tensor_add(
                        ts_[:],
                        psums[ot][:],
                        biasrep[:, ot * NFREE : (ot + 1) * NFREE],
                    )
                    tsums.append(ts_)
                for ot, ts_ in zip(ots, tsums):
                    nc.vector.reduce_max(
                        pooled_t[:, ot, :],
                        ts_[:].rearrange("p (q f) -> p q f", f=KS),
                        axis=mybir.AxisListType.X,
                    )

            for bt in range(BT):
                pooled_t = pooled_pool.tile(
                    [P, OT, P], bf16, tag="pooled", name=f"pooled_{bt}"
                )
                psums = [
                    psum_pool.tile([P, NFREE], f32, tag="ps", name=f"ps_{bt}_{ot}")
                    for ot in range(OT)
                ]
                if bt < BT - 1:
                    # k-pair-major: each W slab is consumed by 2 back-to-back
                    # matmuls; pooling at the tile tail overlaps later tiles'
                    # matmuls on other banks (4-deep bank rotation).
                    for kp in range(KP):
                        if bt == 0 and kp == 0:
                            lhsT = x_first[:]
                        else:
                            lhsT = x_sb[bt][:, kp, :, :]
                        for ot in range(OT):
                            nc.tensor.matmul(
                                psums[ot][:],
                                lhsT,
                                w_sb[kp][:, :, ot * NFREE : (ot + 1) * NFREE],
                                start=(kp == 0),
                                stop=(kp == KP - 1),
                                perf_mode=DR,
                            )
                        if kp == KP - 1:
                            emit_pooling(bt, psums, pooled_t, range(OT))
                else:
                    # Last tile: ot-outer so the first bank's pooling overlaps
                    # the second bank's matmuls, shrinking the tail.
                    for ot in range(OT):
                        for kp in range(KP):
                            nc.tensor.matmul(
                                psums[ot][:],
                                x_sb[bt][:, kp, :, :],
                                w_sb[kp][:, :, ot * NFREE : (ot + 1) * NFREE],
                                start=(kp == 0),
                                stop=(kp == KP - 1),
                                perf_mode=DR,
                            )
                        emit_pooling(bt, psums, pooled_t, [ot])
                nc.vector.reduce_sum(
                    outsb[:, bt : bt + 1],
                    pooled_t[:, :, :],
                    axis=mybir.AxisListType.XY,
                )
            nc.scalar.dma_start(out_d[:, :], outsb[:, :])

    nc.compile()
    _dedup_ldweights(nc)
    return nc


def make_in_maps(x, W, b):
    """Host-side shard + preprocess: transpose, fold 16=0.5*32, cast e4m3."""
    x = np.asarray(x, dtype=np.float32)
    W = np.asarray(W, dtype=np.float32)
    b = np.asarray(b, dtype=np.float32)

    fp8 = ml_dtypes.float8_e4m3

    # Per-batch-group x slabs: [BT, P, KP, 2, P], k = kp*256 + slot*128 + p,
    # batch = bt*128 + m.
    x_slabs = []
    x_firsts = []
    for g in range(NB_G):
        xgT = np.ascontiguousarray(x[g * BC : (g + 1) * BC, :].T).astype(fp8)
        slab = np.ascontiguousarray(
            xgT.reshape(KP, 2, P, BT, P).transpose(3, 2, 0, 1, 4)
        )
        x_slabs.append(slab)
        x_firsts.append(np.ascontiguousarray(slab[0, :, 0, :, :]))
    # Per-out-group W slabs [KP, P, 2, OC] and replicated bias rows.
    w_slabs = []
    b_slabs = []
    for j in range(NO_G):
        wjT = np.ascontiguousarray(
            W[j * OC : (j + 1) * OC, :].T * np.float32(WB_SCALE)
        ).astype(fp8)
        w_slabs.append(
            np.ascontiguousarray(
                wjT.reshape(KP, 2, P, OC).transpose(0, 2, 1, 3)
            )
        )
        bj = (b[j * OC : (j + 1) * OC] * np.float32(WB_SCALE)).astype(
            ml_dtypes.bfloat16
        )
        b_slabs.append(
            np.ascontiguousarray(np.broadcast_to(bj.reshape(1, OC), (P, OC)))
        )

    in_maps = []
    for c in range(NB_G * NO_G):
        g, j = divmod(c, NO_G)
        in_maps.append(
            {
                "xt": x_slabs[g],
                "xt0": x_firsts[g],
                "wt": w_slabs[j],
                "biasrep": b_slabs[j],
            }
        )
    return in_maps


def combine_outputs(results):
    """Sum the 4 out-feature partials per batch half -> full [B] output."""
    out = np.zeros(B, dtype=np.float32)
    for c, r in enumerate(results):
        g = c // NO_G
        part = np.asarray(r["out"], dtype=np.float32)  # [P, BT]
        # batch index within the core = bt*P + p
        out[g * BC : (g + 1) * BC] += part.T.reshape(BC)
    return out * np.float32(1.0 / 32.0)


def kernel(x, W, b):
    from concourse.bass_utils import run_bass_kernel_spmd

    if "nc" not in _NC_CACHE:
        _NC_CACHE["nc"] = build_bass()
    nc = _NC_CACHE["nc"]
    in_maps = make_in_maps(x, W, b)
    res = run_bass_kernel_spmd(nc, in_maps, core_ids=list(range(NB_G * NO_G)))
    return combine_outputs(res.results)
